# revision 1
# baseline (speedup 1.0000x reference)
"""Trainium2 Bass kernel for per-(b,c) WxW attention + residual + BatchNorm + Swish.

Reference math (per (b,c) slice, H=32, W=256):
    S = q^T k          (contract H)        -> [W, W]
    P = softmax(S, axis=-1)
    out = P @ v^T  (-> [H, W] layout)
    x = out + v
    BatchNorm2d over (B, H, W) per channel, then Swish.

Sharding: channels C=64 are split across 8 cores (8 channels each). Each
(b,c) slice is independent and BatchNorm stats are per-channel, so with
C-sharding each core is fully independent — no collectives.

Schedule (quad = 4 consecutive channels of one batch on the 128 SBUF
partitions; matmul operands bf16, accumulation f32):
  - inputs DMA'd 4 batches per transfer (3 DMA dispatches + 1 DVE cast
    per group); the first group is split per-batch to shorten startup;
    the next half's first group is prefetched before this half's pass 2
    so input DMAs are not head-of-line blocked behind output DMAs.
  - MM1 (PE, K=32): c-outer quartets at tile_position (32s, 0) write 4
    DISTINCT psum banks (2 stg tiles x 2 banks) -> true 4-way row
    packing.  stg rotates through 3 slots so the next quad's MM1 never
    waits on this quad's exp (PSUM groups checked partition-oblivious
    by the sim only -> skip_group_check where row-disjoint).
  - exp per g-half [128, 1024] on ACT (the kernel's hard floor:
    64 ops x ~1.1us = ~71us; ACT runs 1 col/cycle @ 1.2 GHz).
  - software pipelining: each iteration issues quad Q's MM1+exp, then
    quad Q-1's back half (V^T transposes, den (ones-weights, 4-way
    col-packed), MM2 (4-way col-packed), den copy + recip + u*rep on
    DVE, residual add on GPSIMD, batched bn_stats).  den + V^T share a
    psum bank (both consumed by early DVE copies); u has its own.
  - channel rstd via DVE bit-trick rsqrt + 2 Newton steps (no Sqrt
    activation-table load); a dependency-pinned dummy Silu prewarms the
    Silu table set during the stats chain.
  - pass 2: one Silu [128, 1024] + one strided output DMA per 4-batch
    group, overlapping the next half's pass 1.
Measured on trn2 (8 cores): ~140-148 us exec, rel err 3.4e-03.
"""

import sys
from contextlib import ExitStack

for _p in ("/opt/trn_rl_repo",):
    if _p not in sys.path:
        sys.path.insert(0, _p)

import numpy as np

import concourse.bacc as bacc
import concourse.bass as bass
import concourse.tile as tile
from concourse import masks, mybir
from concourse.bass_utils import run_bass_kernel_spmd

# Per-core shard shapes (C=64 sharded over 8 cores).
B, C_LOC, H, W = 16, 8, 32, 256
N_CORES = 8
F32 = mybir.dt.float32
BF16 = mybir.dt.bfloat16
F32R = mybir.dt.float32r
BN_EPS = 1e-5


def build_graph():
    nc = bacc.Bacc("TRN2", debug=False, target_bir_lowering=False)

    q_ext = nc.dram_tensor("q", [B, C_LOC, H, W], F32, kind="ExternalInput").ap()
    k_ext = nc.dram_tensor("k", [B, C_LOC, H, W], F32, kind="ExternalInput").ap()
    v_ext = nc.dram_tensor("v", [B, C_LOC, H, W], F32, kind="ExternalInput").ap()
    g_ext = nc.dram_tensor("gamma", [C_LOC], F32, kind="ExternalInput").ap()
    b_ext = nc.dram_tensor("beta", [C_LOC], F32, kind="ExternalInput").ap()
    out_ext = nc.dram_tensor("out", [B, C_LOC, H, W], F32, kind="ExternalOutput").ap()

    with tile.TileContext(nc) as tc:
        with ExitStack() as ctx:
            _build_body(ctx, tc, q_ext, k_ext, v_ext, g_ext, b_ext, out_ext)
    nc.compile()
    return nc


def _build_body(ctx, tc, q_ext, k_ext, v_ext, g_ext, b_ext, out_ext):
    nc = tc.nc
    NHF = C_LOC // 4  # channel-halves ("quads" per batch)

    singles = ctx.enter_context(tc.tile_pool(name="singles", bufs=1))
    qkv = ctx.enter_context(tc.tile_pool(name="qkv", bufs=3))
    bfp = ctx.enter_context(tc.tile_pool(name="bfp", bufs=3))
    pp = ctx.enter_context(tc.tile_pool(name="pp", bufs=4))
    work = ctx.enter_context(tc.tile_pool(name="work", bufs=6))
    x2p = ctx.enter_context(tc.tile_pool(name="x2p", bufs=(B // 4) * NHF))
    yp = ctx.enter_context(tc.tile_pool(name="yp", bufs=4))
    psum = ctx.enter_context(tc.tile_pool(name="psum", bufs=1, space="PSUM"))

    # ---- constants ----
    ident = singles.tile([128, 128], F32, tag="ident")
    masks.make_identity(nc, ident[:])
    ident_bf = singles.tile([128, 128], BF16, tag="ident_bf")
    masks.make_identity(nc, ident_bf[:])

    # ones [128, 32] as denominator-matmul weights: M=32 writes the
    # denominator replicated across each slice's 32 partition rows
    # (PSUM forbids non-unit partition strides, so M=1 rows would be
    # unreadable anyway).
    ones_bf = singles.tile([128, 32], BF16, tag="ones_bf")
    nc.vector.memset(ones_bf[:], 1.0)

    # blk4 [128, 4]: col s = indicator of partition block 32s..32s+32
    blk4 = singles.tile([128, 4], F32, tag="blk4")
    nc.vector.memset(blk4[:], 0.0)
    for s in range(4):
        nc.vector.memset(blk4[32 * s : 32 * (s + 1), s : s + 1], 1.0)
    # blk4T [4, 128]: row s = indicator of columns 32s..32s+32.
    # (Built by transposing blk4 — SBUF writes may only start at
    # partition 0/32/64/96, so per-row memsets at rows 1..3 are illegal.)
    blk4T = singles.tile([4, 128], F32, tag="blk4T")
    blk4T_ps = psum.tile([4, 128], F32, tag="den", bufs=1)
    nc.tensor.matmul(
        out=blk4T_ps[:], lhsT=blk4[:], rhs=ident[:], is_transpose=True,
        start=True, stop=True, skip_group_check=True,
    )
    nc.vector.tensor_copy(blk4T[:], blk4T_ps[:])

    eps_t = singles.tile([4, 1], F32, tag="eps")
    nc.vector.memset(eps_t[:], BN_EPS)

    # gamma/beta: [4, NHF] — partition s = channel-within-half, col = half
    gam = singles.tile([4, NHF], F32, tag="gam")
    bet = singles.tile([4, NHF], F32, tag="bet")
    nc.sync.dma_start(out=gam[:], in_=g_ext.rearrange("(a b) -> b a", b=4))
    nc.sync.dma_start(out=bet[:], in_=b_ext.rearrange("(a b) -> b a", b=4))


    # per-(half, batch) bn stats
    stats = [
        singles.tile([128, (B // 4) * 2, 6], F32, tag=f"stats{hf}", name=f"stats{hf}")
        for hf in range(NHF)
    ]

    x2_tiles = {}

    # tiny scratch for activation-table prewarming (hides the Silu
    # table load inside the stats-chain gap instead of serializing it
    # before the first real Silu)
    dumm = singles.tile([4, 1], F32, tag="dumm")
    nc.vector.memset(dumm[:], 0.0)
    dummo = singles.tile([4, 1], F32, tag="dummo")

    def quad_back_half(pend):
        # V^T + den + MM2 + normalize + residual for a quad whose
        # MM1/exp were already issued (software pipelining: keeps the PE
        # FIFO free to run the NEXT quad's MM1 while its exps finish).
        p_sb = pend["p_sb"]
        v_bf = pend["v_bf"]
        # den and V^T share one bank: both are consumed by early DVE
        # copies, so the WAR for the next quad's transposes resolves
        # early (tying V^T to u's bank would chain it behind x1).
        dv = psum.tile([128, 512], F32, tag="den", bufs=1)
        den_ps = dv[:, 0:256]
        vt_ps = dv[:, 256:384].bitcast(BF16)
        for c in range(2):
            nc.tensor.matmul(
                out=vt_ps[:, 128 * c : 128 * (c + 1)],
                lhsT=v_bf[:, 128 * c : 128 * (c + 1)],
                rhs=ident_bf[:],
                is_transpose=True,
                start=True,
                stop=True,
                skip_group_check=True,
            )
        vt_sb = pp.tile([128, W], BF16, tag="vt_sb")
        nc.vector.tensor_copy(vt_sb[:], vt_ps[:])
        u_ps = psum.tile([128, W], F32, tag="uv", bufs=1)
        for c in range(2):
            for s in range(4):
                nc.tensor.matmul(
                    out=den_ps[32 * s : 32 * (s + 1), :],
                    lhsT=ones_bf[:],
                    rhs=p_sb[:, s * 512 + c * 256 : s * 512 + (c + 1) * 256],
                    start=(c == 0),
                    stop=(c == 1),
                    tile_position=(0, 32 * s),
                    skip_group_check=True,
                )
        for c in range(2):
            for s in range(4):
                nc.tensor.matmul(
                    out=u_ps[32 * s : 32 * (s + 1), :],
                    lhsT=vt_sb[:, 128 * c + 32 * s : 128 * c + 32 * (s + 1)],
                    rhs=p_sb[:, s * 512 + c * 256 : s * 512 + (c + 1) * 256],
                    start=(c == 0),
                    stop=(c == 1),
                    tile_position=(0, 32 * s),
                    skip_group_check=True,
                )

        # recip_approx reads its input twice -> PSUM source is illegal,
        # so stage in SBUF first (copy runs at 2x from PSUM).
        den_sb = work.tile([128, W], F32, tag="den_sb")
        nc.vector.tensor_copy(den_sb[:], den_ps[:])
        rep = work.tile([128, W], F32, tag="rep")
        nc.vector.reciprocal_approx_fast(out=rep[:], in_=den_sb[:])

        x1 = work.tile([128, W], F32, tag="x1")
        nc.vector.tensor_mul(x1[:], u_ps[:], rep[:])

        nc.gpsimd.tensor_add(pend["xsl"], x1[:], pend["vQ"])

        hf, b = pend["hf"], pend["b"]
        if b % 4 == 3:
            x2 = x2_tiles[(b // 4, hf)]
            for hb in range(2):
                nc.vector.bn_stats(
                    out=stats[hf][:, 2 * (b // 4) + hb, :],
                    in_=x2[:, hb * 2 * W : (hb + 1) * 2 * W],
                )

    prefetched = {}

    def load_group(hf, bb, split):
        # DMA q,k,v for 4 batches + bf16 cast. split=True issues
        # per-batch DMAs/casts so the very first quad starts sooner.
        qkv_g = qkv.tile([128, 3, 4, W], F32, tag="qkv_t")
        qkv_bfg = bfp.tile([128, 3, 4, W], BF16, tag="qkv_bf")
        if split:
            for jj in range(4):
                for ti, src_t in enumerate((q_ext, k_ext, v_ext)):
                    nc.sync.dma_start(
                        out=qkv_g[:, ti, jj],
                        in_=src_t[
                            4 * bb + jj, 4 * hf : 4 * hf + 4
                        ].rearrange("c h w -> (c h) w"),
                    )
                nc.vector.tensor_copy(qkv_bfg[:, :, jj], qkv_g[:, :, jj])
        else:
            for ti, src_t in enumerate((q_ext, k_ext, v_ext)):
                nc.sync.dma_start(
                    out=qkv_g[:, ti],
                    in_=src_t[
                        4 * bb : 4 * bb + 4, 4 * hf : 4 * hf + 4
                    ].rearrange("b c h w -> (c h) b w"),
                )
            nc.vector.tensor_copy(
                qkv_bfg.rearrange("p a b w -> p (a b w)"),
                qkv_g.rearrange("p a b w -> p (a b w)"),
            )
        return qkv_g, qkv_bfg

    # ---------------- pass 1 (hf-major: half 0's stats + pass 2 can
    # overlap half 1's pass 1) ----------------
    for hf in range(NHF):
        pend = None
        for b in range(B):
            if b % 4 == 0:
                bb = b // 4
                if (hf, bb) in prefetched:
                    qkv_g, qkv_bfg = prefetched.pop((hf, bb))
                else:
                    qkv_g, qkv_bfg = load_group(hf, bb, split=(hf == 0 and bb == 0))
                x2_tiles[(bb, hf)] = x2p.tile(
                    [128, 4 * W], F32, tag="x2", name=f"x2_{bb}_{hf}"
                )
            j = b % 4
            vQ = qkv_g[:, 2, j]
            q_bf = qkv_bfg[:, 0, j]
            k_bf = qkv_bfg[:, 1, j]
            v_bf = qkv_bfg[:, 2, j]

            # MM1: S^T[v, w] per slice; half g holds slices {2g, 2g+1},
            # slice jj chunk c at free offset jj*512 + c*256.
            # stg rotates through 3 one-quad-half slots so the next
            # quad's MM1 never waits on this quad's exp; the two
            # concurrent row-tiles write the slot's 2 distinct banks.
            p_sb = pp.tile([128, 4 * 512], BF16, tag="p_sb")
            stg_a = psum.tile([128, 2 * 512], F32, tag="st", bufs=3)
            stg_b = psum.tile([128, 2 * 512], F32, tag="st", bufs=3)
            stg_g = [stg_a, stg_b]
            # c-outer quartets: the 4 concurrent row-tiles land in 4
            # DISTINCT psum banks (2 per stg tile) -> true 4-way packing
            for c in range(2):
                for s in range(4):
                    nc.tensor.matmul(
                        out=stg_g[s // 2][
                            :, (s % 2) * 512 + c * 256 : (s % 2) * 512 + (c + 1) * 256
                        ],
                        lhsT=k_bf[32 * s : 32 * (s + 1), 128 * c : 128 * (c + 1)],
                        rhs=q_bf[32 * s : 32 * (s + 1), :],
                        start=True,
                        stop=True,
                        tile_position=(32 * s, 0),
                    )
            for g in range(2):
                nc.scalar.activation(
                    p_sb[:, g * 1024 : (g + 1) * 1024],
                    stg_g[g][:],
                    mybir.ActivationFunctionType.Exp,
                )

            if pend is not None:
                quad_back_half(pend)
            x2 = x2_tiles[(b // 4, hf)]
            pend = {
                "p_sb": p_sb,
                "v_bf": v_bf,
                "vQ": vQ,
                "xsl": x2[:, j * W : (j + 1) * W],
                "hf": hf,
                "b": b,
            }
        quad_back_half(pend)

        nc.scalar.activation(
            out=dummo[:],
            in_=stats[hf][0:4, 2 * (B // 4) - 1, 0:1],
            func=mybir.ActivationFunctionType.Silu,
        )

        if hf + 1 < NHF:
            prefetched[(hf + 1, 0)] = load_group(hf + 1, 0, split=False)

        # ------- channel statistics + pass 2 for this half -------
        mv = work.tile([128, 2], F32, tag="mv")
        nc.vector.bn_aggr(out=mv[:], in_=stats[hf][:])
        t3 = work.tile([128, 3], F32, tag="t3")
        nc.vector.tensor_copy(t3[:, 0:2], mv[:])
        nc.vector.tensor_mul(t3[:, 2:3], mv[:, 0:1], mv[:, 0:1])

        chn = psum.tile([4, 3], F32, tag="den", bufs=1)
        nc.tensor.matmul(
            out=chn[:], lhsT=blk4[:], rhs=t3[:], start=True, stop=True,
            skip_group_check=True,
        )
        # stage in SBUF (only one PSUM input allowed per instruction)
        chn_sb = work.tile([4, 3], F32, tag="chn_sb")
        nc.vector.tensor_copy(chn_sb[:], chn[:])
        # mean_c = chn[:,0]/32 ; var_c = (chn[:,1]+chn[:,2])/32 - mean_c^2
        m_c = work.tile([4, 1], F32, tag="m_c")
        nc.vector.tensor_scalar_mul(m_c[:], chn_sb[:, 0:1], 1.0 / 32.0)
        msq = work.tile([4, 1], F32, tag="msq")
        nc.vector.tensor_mul(msq[:], m_c[:], m_c[:])
        vsum = work.tile([4, 1], F32, tag="vsum")
        nc.vector.tensor_add(vsum[:], chn_sb[:, 1:2], chn_sb[:, 2:3])
        var_c = work.tile([4, 1], F32, tag="var_c")
        nc.vector.scalar_tensor_tensor(
            out=var_c[:],
            in0=vsum[:],
            scalar=1.0 / 32.0,
            in1=msq[:],
            op0=mybir.AluOpType.mult,
            op1=mybir.AluOpType.subtract,
        )
        # rstd = 1/sqrt(var+eps) via DVE bit-trick + Newton (avoids the
        # Sqrt activation-table load on the scalar engine)
        varep = work.tile([4, 1], F32, tag="varep")
        nc.vector.tensor_scalar_add(varep[:], var_c[:], BN_EPS)
        y0i = work.tile([4, 1], mybir.dt.int32, tag="y0i")
        nc.vector.tensor_scalar(
            y0i[:],
            varep.bitcast(mybir.dt.int32),
            1,
            -1,
            op0=mybir.AluOpType.arith_shift_right,
            op1=mybir.AluOpType.bitwise_xor,
        )
        nc.vector.tensor_scalar_add(y0i[:], y0i[:], 0x5F3759E0)
        rstd = y0i.bitcast(F32)
        tnr = work.tile([4, 1], F32, tag="tnr")
        for _ in range(2):
            nc.vector.tensor_mul(tnr[:], rstd, rstd)
            nc.vector.tensor_mul(tnr[:], tnr[:], varep[:])
            nc.vector.tensor_scalar(
                tnr[:],
                tnr[:],
                -0.5,
                1.5,
                op0=mybir.AluOpType.mult,
                op1=mybir.AluOpType.add,
            )
            nc.vector.tensor_mul(rstd, rstd, tnr[:])
        sc_c = work.tile([4, 1], F32, tag="sc_c")
        nc.vector.tensor_mul(sc_c[:], gam[:, hf : hf + 1], rstd)
        ms = work.tile([4, 1], F32, tag="ms")
        nc.vector.tensor_mul(ms[:], m_c[:], sc_c[:])
        sh_c = work.tile([4, 1], F32, tag="sh_c")
        nc.vector.tensor_sub(sh_c[:], bet[:, hf : hf + 1], ms[:])

        # replicate [4,1] -> [128,1] (each value over its 32-partition block)
        screp_ps = psum.tile([128, 1], F32, tag="den")
        nc.tensor.matmul(
            out=screp_ps[:], lhsT=blk4T[:], rhs=sc_c[:], start=True, stop=True,
            skip_group_check=True,
        )
        screp = singles.tile([128, 1], F32, tag=f"screp{hf}")
        nc.vector.tensor_copy(screp[:], screp_ps[:])
        shrep_ps = psum.tile([128, 1], F32, tag="den")
        nc.tensor.matmul(
            out=shrep_ps[:], lhsT=blk4T[:], rhs=sh_c[:], start=True, stop=True,
            skip_group_check=True,
        )
        shrep = singles.tile([128, 1], F32, tag=f"shrep{hf}")
        nc.vector.tensor_copy(shrep[:], shrep_ps[:])

        # ------- pass 2 for this half: Silu/store in 2-batch chunks so
        # the output DMA overlaps the next chunk's activation -------
        for bb in range(B // 4):
            x2 = x2_tiles[(bb, hf)]
            y = yp.tile([128, 4 * W], F32, tag="y")
            nc.scalar.activation(
                out=y[:],
                in_=x2[:],
                func=mybir.ActivationFunctionType.Silu,
                bias=shrep[:],
                scale=screp[:],
            )
            nc.sync.dma_start(
                out=out_ext[
                    4 * bb : 4 * (bb + 1), 4 * hf : 4 * hf + 4
                ].rearrange("b c h w -> (c h) b w"),
                in_=y.rearrange("p (b w) -> p b w", b=4),
            )



_NC_CACHE = None


def kernel(query, key, value, gamma, beta):
    global _NC_CACHE
    query = np.ascontiguousarray(np.asarray(query, dtype=np.float32))
    key = np.ascontiguousarray(np.asarray(key, dtype=np.float32))
    value = np.ascontiguousarray(np.asarray(value, dtype=np.float32))
    gamma = np.ascontiguousarray(np.asarray(gamma, dtype=np.float32))
    beta = np.ascontiguousarray(np.asarray(beta, dtype=np.float32))

    if _NC_CACHE is None:
        _NC_CACHE = build_graph()
    nc = _NC_CACHE

    in_maps = []
    for i in range(N_CORES):
        cs = slice(i * C_LOC, (i + 1) * C_LOC)
        in_maps.append(
            {
                "q": np.ascontiguousarray(query[:, cs]),
                "k": np.ascontiguousarray(key[:, cs]),
                "v": np.ascontiguousarray(value[:, cs]),
                "gamma": np.ascontiguousarray(gamma[cs]),
                "beta": np.ascontiguousarray(beta[cs]),
            }
        )

    res = run_bass_kernel_spmd(nc, in_maps, core_ids=list(range(N_CORES)))
    out = np.empty((B, N_CORES * C_LOC, H, W), dtype=np.float32)
    for i in range(N_CORES):
        out[:, i * C_LOC : (i + 1) * C_LOC] = res.results[i]["out"]
    return out


if __name__ == "__main__":
    g = build_graph()
    print("graph built OK")

